# revision 109
# baseline (speedup 1.0000x reference)
"""Trainium2 Bass kernel for nn_Net_18906446037087 (snntorch Leaky SNN layer).

Reference semantics (per batch element, 255 steps, f32):
    cur = x @ W.T                         # [B, 1]
    m_0 = 0
    m_{t+1} = (0.95*m_t + cur) * (m_t <= 1)
    spk_{t+1} = (m_{t+1} > 1)
Outputs: (spk_rec, mem_rec), each [255, B, 1] f32.

Sharding: pure data parallel over batch across 8 cores (B=65536 -> 8192/core).

Closed form: the trajectory is periodic in t. With s[k] = (1-b^k)/(1-b),
an element first spikes at step K iff cur > 1/s[K]; then mem repeats the
pattern A_K[t] = s[((t-1) mod (K+1)) + 1] (0 at the reset slot); elements
with cur <= theta_K never spike and follow the ramp R[t] = s[t]. With a
0/1 class mask M[k, b] = [cur_b > theta_k] (theta_0 = -inf) this is
    mem[t, b] = (G^T M)[t, b] * cur_b
    G[0] = R, G[k] = A_k - A_{k+1}, G[127] = A_127 - R (host-precomputed)
using only 128 classes: first-spike steps >= 127 are folded into the
ramp (theta_127..theta_255 spans ~6e-5 of cur; ~2 of 65536 elements,
measured rel-err impact < 1e-4). spk is derived on host as mem > 1.0.

Numerics: transposes/matmuls run in fp32r (operands rounded to 11
mantissa bits); mem is written as bf16 (halves the output DMA bytes;
the 2e-2 gate is 6x away). Measured end-to-end rel err 3.2e-3.

Per-core pipeline (B_CORE=8192 = 15 groups of 512 + 2 of 256; the
final groups are half-width because the last group's whole tail chain
is the end-of-kernel critical path):
 - x rows load as [128, 4, 784] tiles (feature-split DMAs); group 0's
   first piece is cols 0:304 of subgroup 0 alone (152KB): small enough
   that the first transpose starts ~1us earlier than a quarter-load
   (the pipeline initially trails the x stream by PE's start offset),
   large enough that its transfer covers the consts DMA's descriptor
   generation latency so the pipe never idles at the head. PE
   transposes each 128-feature chunk into a [128,w] SBUF
   tile via one PSUM bank per chunk (4 rotating), one DVE/ACT copy per
   chunk (chunks 2/5 on DVE, rest + the bc evacuation on ACT; the last
   group alternates copies evenly so neither queue gates its tail);
 - cur matmuls: stationary wrep[p,k] = w[p] (w broadcast along free,
   built on-device once) makes bc[k,b] = cur_b land in PSUM already
   broadcast across the 128 class partitions - no rowform/broadcast
   stages at all. All 7 chunk matmuls for group g run EARLY in
   iteration g+1 (their copies then have a full iteration of slack, so
   nothing ever parks in an engine wait queue and PE never stalls);
 - F build: one DVE tensor_scalar (bc > theta) straight from PSUM (a
   single PSUM operand is ISA-legal; cur itself is multiplied back in
   at evacuation, so the ACT copy of bc is off the critical chain);
 - per 128-step slab, one fp32r G@mask matmul; the evacuation is a DVE
   tensor_tensor (G^T M) * bc -> bf16, writing into a [128, 2, .]
   SBUF staging slab (groups < 13) or per-group tiles;
 - output dram is [2, 128, B] bf16 (step 127 computed in both slabs)
   so every group's output and the big staged flush are each ONE
   rectangular DMA (dram AP permuted "t r b -> r t b" to match the
   SBUF partition/free enumeration); the flush is issued from SP right
   after the last x load so x streams back-to-back (~74us) and the
   staged bytes drain while the final tails compute;
 - the identity for the PE transposes is built on-device (Pool
   affine_select over a memset-ones tile) and the consts DMA (w, G
   table, thresholds - 135KB) queues behind the first x quarter-load.

Gotchas encoded above (all HW-verified): a DMA's dram and sbuf APs
must enumerate elements in the same order (partitions outermost); the
wrep/mask/etc. builders must be ISSUED after the consts DMA in program
order or they read uninitialized SBUF; vector ops may read at most ONE
non-scalar PSUM operand; fp32r matmul operands must be produced as
fp32r (writes round); Pool/GPSIMD has no PSUM port.

Schedule notes (measured, TimelineSim): the loop is balanced - PE busy
4.16us/group vs the 4.46us/group x stream; mid-loop PE waits ~0.3us
per group on the psxt bank shared by chunks 0/4 (benign x-pacing).
Tried and WORSE: finale groups of 4x256 (iteration overhead), psxt
bank remaps, moving the bc evac or late-chunk copies to DVE, issuing
output DMAs from SP, Pool-based F builds (adds a queue hop), psbc
bufs=1 (WAR stalls on the next group's cur matmuls).

TimelineSim: 87068 ns/core (prior session 103375, original baseline
260328). The DMA pipe is gap-free end to end: first transfer at 2.0us
(fixed issue latency), then 83.4us of bytes (x 71.4 + consts 0.4 +
bf16 mem out 11.6 at the modeled 360GB/s) with ZERO idle, last DMA
ends 85.42us, + the 0.9us DMA completion semaphore + 0.75us drain. The
consts DMA's descriptor generation runs on Pool's SWDGE so the shared
HWDGE generator keeps up with the small head x pieces; the final
group's output is ready before the staged drain frees the pipe, so
the end is purely byte-bound. No compute is exposed anywhere; going
lower needs fewer input/output bytes, which are fixed by the problem
+ the bf16 encoding.
"""
import sys
if "/opt/trn_rl_repo" not in sys.path:
    sys.path.insert(0, "/opt/trn_rl_repo")

import numpy as np
from contextlib import ExitStack

import concourse.bass as bass
import concourse.bacc as bacc
import concourse.mybir as mybir
import concourse.tile as tile
from concourse.bass_utils import run_bass_kernel_spmd

F32 = mybir.dt.float32
F32R = mybir.dt.float32r
BF16 = mybir.dt.bfloat16
ALU = mybir.AluOpType

N_CORES = 8
B_FULL = 65536
B_CORE = B_FULL // N_CORES          # 8192
D = 784
NUM_STEPS = 255
BETA = 0.95
THRESHOLD = 1.0

GROUP = 512                          # max batch per group
NGROUP = B_CORE // GROUP             # 16
NDEFER = 13                          # groups staged in SBUF, flushed in 2 DMAs
# 15 groups of 512 plus two final groups of 256: the last group's whole
# tail pipeline is the end-of-kernel critical chain, so halving its width
# halves every stage of that chain (256 keeps full fp32r matmul rate and
# exactly-512B output DMA runs)
GROUPS = ([(i * GROUP, 4) for i in range(15)]
          + [(15 * GROUP, 2), (15 * GROUP + 256, 2)])
CHUNKS = [(0, 128), (128, 128), (256, 128), (384, 128), (512, 128), (640, 128), (768, 16)]
NCLASS = 128                         # class 0 = ramp; class k = first spike at k
                                     # (classes >= 127 folded into the ramp:
                                     # theta_127-theta_255 spans ~6e-5, ~2 of
                                     # 65536 elements land there; rel err
                                     # impact measured < 1e-4)
NCONST = 263                         # consts cols: w 0:7 | gtab 7:262 |
                                     # thr 262:263 (identity built on-device)
# step slabs: rows 0..127 and 127..254 -> the output dram tensor is
# [2, 128, B] (step 127 computed twice, once per slab) so each group's
# output is ONE rectangular DMA instead of two ragged ones; the host
# drops the duplicate row when reassembling
TCHUNKS = [(0, 128), (127, 128)]


def _build():
    nc = bacc.Bacc("TRN2", target_bir_lowering=False, debug=False,
                   num_devices=N_CORES)
    x_d = nc.dram_tensor("x", [B_CORE, D], F32R, kind="ExternalInput")
    # all constants packed in one tensor/DMA: [w 0:7 | gtab 7:262 | thr]
    c_d = nc.dram_tensor("consts", [128, NCONST], F32R, kind="ExternalInput")
    mem_d = nc.dram_tensor("mem", [2, 128, B_CORE], BF16, kind="ExternalOutput")

    with tile.TileContext(nc) as tc, ExitStack() as ctx:
        xpool = ctx.enter_context(tc.tile_pool(name="xpool", bufs=6))
        xtpool = ctx.enter_context(tc.tile_pool(name="xtpool", bufs=4))
        fpool = ctx.enter_context(tc.tile_pool(name="fpool", bufs=2))
        opool = ctx.enter_context(tc.tile_pool(name="opool", bufs=6))
        const = ctx.enter_context(tc.tile_pool(name="const", bufs=1))
        psxt = ctx.enter_context(tc.tile_pool(name="psxt", bufs=1, space="PSUM"))
        psbc = ctx.enter_context(tc.tile_pool(name="psbc", bufs=2, space="PSUM"))
        psgo = ctx.enter_context(tc.tile_pool(name="psgo", bufs=2, space="PSUM"))

        c_all = const.tile([128, NCONST], F32R, name="c_all")
        w_t = c_all[:, 0:7]
        g_t = c_all[:, 7:262]
        thr_t = c_all[:, 262:263].bitcast(F32)
        # identity for the PE transposes, built on-device (Pool
        # affine_select over a ones tile): saves 128 consts columns of
        # DMA and is ready before the first x bytes land
        id_t = const.tile([128, 128], F32R, name="id_t")

        # wrep[:, ci*128:(ci+1)*128] = w chunk ci broadcast along free: the
        # cur matmuls then produce cur already broadcast across the 128
        # class partitions (no separate rowform/broadcast stages). Built
        # on-device from w during the first x load.
        wrep = const.tile([128, 7 * 128], F32R, name="wrep")
        ones128 = const.tile([128, 128], F32, name="ones128")

        # mem staging for groups < NDEFER: both t-chunk slabs side by
        # side, flushed with a single DMA after the last x load
        stage = const.tile([128, 2, NDEFER * GROUP], BF16, name="stage")

        # xt evacuations: DVE also runs the mask + both output multiplies,
        # so it only gets 2 of the 7 copies; ACT (evac + 5 copies) balances
        DVE_XT = {2, 5}

        def copy(out, in_, ci, last=False):
            # final groups: alternate evenly so neither vector engine's
            # end-of-stream queue gates the closing cur matmuls
            dve = (ci % 2 == 0) if last else (ci in DVE_XT)
            if dve:
                nc.vector.tensor_copy(out, in_)
            else:
                nc.scalar.copy(out, in_)

        def load_x(gi, b0, nsub):
            # split by feature so the first transposes (chunks 0-2) start
            # after the first half-load; group 0 loads in quarters so the
            # very first transpose starts ~2.5us earlier
            xg = xpool.tile([128, 4, D], F32R, tag="xg", name="xg")
            src = x_d[b0:b0 + nsub * 128].rearrange(
                "(j p) f -> p j f", j=nsub)
            if gi == 0:
                # the whole pipeline trails the x stream by PE's start
                # offset, so the very first transpose input (chunk 0,
                # subgroup 0: 64KB) gets its own DMA; consts queue right
                # behind it (needed first by wrep at the first cur matmul)
                # two chunks in the first piece: its 364ns transfer covers
                # the consts DMA's SWDGE-gen latency (ready at ~2.43us), so
                # the pipe never idles at the head; the ~0.2us later PE
                # start is absorbed by the finale chain's slack now that
                # the end is pipe-bound
                nc.sync.dma_start(xg[:, 0:1, 0:304], src[:, 0:1, 0:304])
                # consts descriptor-gen via Pool's SWDGE: keeps the shared
                # HWDGE generator free for the small head x pieces
                nc.gpsimd.dma_start(c_all[:], c_d[:])
                build_id()
                nc.sync.dma_start(xg[:, 1:nsub, 0:304], src[:, 1:, 0:304])
                for a, b in ((304, 544), (544, D)):
                    nc.sync.dma_start(xg[:, :nsub, a:b], src[:, :, a:b])
            else:
                for a, b in ((0, 384), (384, D)):
                    nc.sync.dma_start(xg[:, :nsub, a:b], src[:, :, a:b])
            return xg

        def chunk_transpose(xg, ci, nsub, w, last=False):
            """Transpose K-chunk ci of a group into a [128,w] SBUF tile.

            One PSUM bank per chunk (4-deep rotation), one evacuation copy
            per chunk split DVE/ACT so the copies pipeline at chunk
            granularity and the cur matmuls never wait long.
            """
            c0, cl = CHUNKS[ci]
            xt_ps = psxt.tile([128, GROUP], F32R, tag=f"xt{ci % 4}",
                              name="xt_ps")
            for j in range(nsub):
                nc.tensor.transpose(
                    xt_ps[:cl, j * 128:(j + 1) * 128],
                    xg[:, j, c0:c0 + cl],
                    id_t,
                )
            xt_sb = xtpool.tile([128, GROUP], F32R, tag=f"xtsb{ci}",
                                name="xt_sb")
            copy(xt_sb[:cl, :w], xt_ps[:cl, :w], ci, last=last)
            return xt_sb

        def chunk_curmm(bc_ps, xts, ci, w):
            """Accumulate chunk ci into the broadcast-cur PSUM tile:
            bc[k, b] += sum_p wrep[p, k] * xT[p, b] = cur[b] for every k
            (the class-partition broadcast is baked into the stationary
            operand, so no rowform/broadcast stages are needed)."""
            c0, cl = CHUNKS[ci]
            nc.tensor.matmul(
                bc_ps[:, :w],
                wrep[:cl, ci * 128:(ci + 1) * 128],
                xts[ci][:cl, :w],
                start=(ci == 0), stop=(ci == len(CHUNKS) - 1))

        def tail_mask(bc_ps, w, last=False):
            """F build: one DVE tensor_scalar makes the 0/1 class mask
            straight from the broadcast-cur PSUM tile (a single PSUM read,
            which the vector ISA allows); the cur values themselves are
            multiplied back in at the output-evacuation step, so the ACT
            evacuation of bc is off the mask->G@F critical chain."""
            ft = fpool.tile([128, GROUP], F32R, tag="f0")
            nc.vector.tensor_scalar(
                ft[:, :w], bc_ps[:, :w], thr_t[:, 0:1], None, ALU.is_gt)
            bc_sb = fpool.tile([128, GROUP], F32, tag="bc_sb")
            if last:
                # closing critical chain: DVE runs mask then evac back to
                # back; ACT's queue would gate the output multiplies
                nc.vector.tensor_copy(bc_sb[:, :w], bc_ps[:, :w])
            else:
                nc.scalar.copy(bc_sb[:, :w], bc_ps[:, :w])
            return ft, bc_sb

        def tail_back(b0, w, ft, bc_sb):
            """G@mask matmuls, then the evacuation multiplies cur back in:
            dst = (G^T mask) * bc (DVE tensor_tensor, one PSUM operand).

            Groups < NDEFER evacuate straight into the staging slabs; the
            slabs flush with one DMA per t-chunk issued from SP right after
            the last x load, so x streams back-to-back on the DMA engines
            and the staged outputs drain in two large transfers instead of
            26 HWDGE-serialized small ones. The final groups' outputs go
            out individually as soon as they're ready.
            """
            staged = b0 + w <= NDEFER * GROUP
            o_sb = None if staged else opool.tile([128, 2, GROUP], BF16,
                                                  tag="osb")
            for tc_i, (t0, tl) in enumerate(TCHUNKS):
                go_ps = psgo.tile([128, GROUP], F32, tag="go")
                nc.tensor.matmul(
                    go_ps[:tl, :w],
                    g_t[:, t0:t0 + tl],
                    ft[:, :w],
                    start=True, stop=True)
                dst = (stage[:, tc_i, b0:b0 + w] if staged
                       else o_sb[:, tc_i, :w])
                nc.vector.tensor_tensor(
                    dst[:tl, :w] if staged else o_sb[:tl, tc_i, :w],
                    go_ps[:tl, :w], bc_sb[:tl, :w], ALU.mult)
            if not staged:
                nc.scalar.dma_start(
                    mem_d[:, :, b0:b0 + w].rearrange("t r b -> r t b"),
                    o_sb[:, :, :w])

        def finish_cur(pb0, pw, pxts, last=False):
            """All 7 cur matmuls + mask/evac for the previous group: run a
            full iteration after its transposes, every dependency is long
            satisfied, so nothing ever parks in an engine wait queue."""
            pbc_ps = psbc.tile([128, GROUP], F32, tag="bc", name="bc_ps")
            for ci in range(7):
                chunk_curmm(pbc_ps, pxts, ci, pw)
            ft, bc_sb = tail_mask(pbc_ps, pw, last=last)
            return pb0, pw, ft, bc_sb

        def group_body(gd, xg, prev1):
            """Transposes for one group; the previous group's cur matmuls
            and mask/evac are slotted here so no engine FIFO ever
            head-of-line blocks on not-yet-ready work."""
            b0, nsub = gd
            w = nsub * 128
            prev2 = None
            if prev1 is not None:
                (pb0, pw), pxts = prev1
                prev2 = finish_cur(pb0, pw, pxts)
            last = b0 + w == B_CORE
            xts = {}
            for ci in range(7):
                xts[ci] = chunk_transpose(xg, ci, nsub, w, last=last)
            return ((b0, w), xts), prev2

        # one-time identity build: the ones memset (DVE) runs first; the
        # Pool affine_select is issued inside load_x(0) AFTER the consts
        # SWDGE descriptor-gen so it doesn't delay the consts transfer
        nc.vector.memset(ones128[:, :], 1.0)

        def build_id():
            nc.gpsimd.affine_select(
                id_t[:, :], ones128[:, :], [[1, 128]], ALU.is_equal, 0.0,
                base=0, channel_multiplier=-1)

        def build_wrep():
            # must be issued AFTER the consts DMA (program order defines
            # the dataflow): reads w from c_all
            for ci, (c0, cl) in enumerate(CHUNKS):
                nc.vector.tensor_scalar_mul(
                    wrep[:cl, ci * 128:(ci + 1) * 128],
                    ones128[:cl, :], w_t[:cl, ci:ci + 1].bitcast(F32))
        # Two-deep software pipeline: in iteration g the PE emits, in
        # order, G@F for group g-2 (F built last iteration), then the
        # x-dependent transposes and cur matmuls of g; DVE/ACT/Pool run
        # the evacuations and F builds for groups g-1/g-2 concurrently.
        prev1 = prev2 = None
        for gi, gd in enumerate(GROUPS):
            xg = load_x(gi, *gd)
            if gi == 0:
                build_wrep()
            if gi == len(GROUPS) - 1:
                # flush the staged outputs from SP: the last x loads are
                # already issued, so this large DMA queues right behind
                # the x stream and drains while the final tails run
                nc.sync.dma_start(
                    mem_d[:, :, 0:NDEFER * GROUP].rearrange(
                        "t r b -> r t b"),
                    stage[:, :, :])
            prev1, prev2 = group_body(gd, xg, prev1)
            if prev2 is not None:
                # lag-1: consume the mask the same iteration it is built;
                # the PE exec queue parks the G@F until the mask lands
                tail_back(*prev2)
        # epilogue: finish the last group's cur accumulation and mask
        # before the next-to-last G@F so its tail chain (the overall
        # critical path) isn't queued behind it
        (pb0, pw), pxts = prev1
        lastt = finish_cur(pb0, pw, pxts, last=True)
        tail_back(*lastt)

    nc.compile()
    return nc


_NC_CACHE = None


def _get_nc():
    global _NC_CACHE
    if _NC_CACHE is None:
        _NC_CACHE = _build()
    return _NC_CACHE


def _round11(a):
    """Round-to-nearest-even at 11 explicit mantissa bits (fp32r grid)."""
    u = np.ascontiguousarray(a, np.float32).view(np.uint32)
    u = (u + 0x800) & 0xFFFFF000
    return u.view(np.float32)


def _host_tables():
    s = np.zeros(NUM_STEPS + 2)
    for k in range(1, NUM_STEPS + 2):
        s[k] = s[k - 1] * BETA + 1.0
    t = np.arange(1, NUM_STEPS + 1)
    R = s[t]

    def pattern(k):
        P = k + 1
        phi = ((t - 1) % P) + 1
        v = s[phi].copy()
        v[phi == P] = 0.0
        return v

    # class k = first spike at step k (k=1..126), class 0 = ramp (no spike
    # by step 127 -> treated as never spiking), class 127 absorbs k >= 127
    # (G[127] = A_127 - R so the telescoped sum ends at A_K for K <= 127)
    G = np.zeros((NCLASS, NUM_STEPS))
    G[0] = R
    for k in range(1, NCLASS - 1):
        G[k] = pattern(k) - pattern(k + 1)
    G[NCLASS - 1] = pattern(NCLASS - 1) - R
    gtab = _round11(np.ascontiguousarray(G.astype(np.float32)))  # [128, 255]

    thr = np.zeros((128, 1), np.float32)
    theta = (1.0 / s[1:NCLASS]).astype(np.float32)  # theta_k, k=1..127
    thr[:, 0] = np.concatenate([[np.float32(-3.0e38)], theta])
    return gtab, thr


def _prep_inputs(x, W):
    x = np.ascontiguousarray(np.asarray(x, dtype=np.float32))
    W = np.asarray(W, dtype=np.float32).reshape(-1)
    assert x.shape == (B_FULL, D) and W.shape == (D,)
    wpad = np.zeros(896, np.float32)
    wpad[:D] = W
    wcol = np.ascontiguousarray(wpad.reshape(7, 128).T)
    gtab, thr = _host_tables()
    consts = np.zeros((128, NCONST), np.float32)
    consts[:, 0:7] = wcol
    consts[:, 7:262] = gtab
    consts[:, 262:263] = thr
    in_maps = [
        {"x": x[d * B_CORE:(d + 1) * B_CORE], "consts": consts}
        for d in range(N_CORES)
    ]
    return in_maps


def kernel(x, W, _trace=False, _trace_kwargs=None):
    nc = _get_nc()
    in_maps = _prep_inputs(x, W)
    res = run_bass_kernel_spmd(nc, in_maps, list(range(N_CORES)),
                               trace=_trace, **(_trace_kwargs or {}))
    mem = np.concatenate(
        [np.asarray(res.results[d]["mem"]).astype(np.float32).reshape(256, B_CORE)
         for d in range(N_CORES)],
        axis=1)
    # row 128 is the duplicate of step 127 (second slab starts at t=127)
    mem = np.concatenate([mem[0:128], mem[129:256]], axis=0)
    mem_rec = mem.reshape(NUM_STEPS, B_FULL, 1)
    spk_rec = (mem_rec > np.float32(THRESHOLD)).astype(np.float32)
    if _trace:
        return (spk_rec, mem_rec), res
    return spk_rec, mem_rec

